# revision 74
# baseline (speedup 1.0000x reference)
"""Cross-attention Bass/Tile kernel for Trainium2, data-parallel over batch on 8 cores.

Problem (hardcoded): x_1 [2048,16,100], x_2 [2048,16,100], Wq/Wk/Wv [100,128], fp32.
  Q = x1 @ Wq; K = x2 @ Wk; V = x2 @ Wv  (per batch)
  out = softmax(Q K^T / sqrt(128)) @ V   -> [2048,16,128]

Sharding: batch dim split 8 ways (2 batches per core). Full inputs in, full output out.

Design notes (ACT-engine paced; ~104us vs 121us baseline):
  The per-core floor is the scalar/ACT engine: 2*S*S = 8.4M exps at 1 elem/lane/cycle
  @1.2GHz + ~260ns/instr bubble -> ~71.5us for 64 [128,1024] EXPs (measured 1117ns
  cadence). PE issue work (S^T + PV + prep) is ~65us. So ACT does NOTHING but the
  exps, and every other engine/queue is scheduled around keeping that stream dense:
   - hardware facts (trace-measured): LDWEIGHTS fully overlaps matmuls; bf16 matmul
     issues 1 col/cycle @2.4GHz (+163ns drain in reported durations); fp32
     transpose runs LOW_HIGH at ~1/4 rate (so transposes stay bf16); Pool/gpsimd
     cannot access PSUM and runs tensor ops at ~0.5 elem/cycle/lane; DVE 2x mode
     needs all-2-byte APs; DMA is packet-rate limited: 800B in-packets give
     ~40-60GB/s per queue stream, 512B out-packets ~25GB/s.
   - input: x2 (all of it) + x1 g0 gate chunk 0, ~2MB > queue rate, so pairs of
     t-tiles are deadline-scheduled across sync/scalar (fp32 staging + DVE casts
     interleaved into the prep chain) and gpsimd DGE-cast streams.
   - PSUM evictions (xT/qT/kT/V/O^T, all psum->SBUF) and tail muls live on DVE;
     rowsum tree: L1/L2 on DVE (2x, 16-bit), short L3/L4 on Pool; chunks 6-7 use
     an incremental DVE tree so the drain tails are not gated on Pool.
   - per chunk: S^T pair [128,1024] fp32 psum (2 matmuls) -> ACT exp*scale -> et
     bf16 SBUF; PV of the previous chunk interleaved 2-matmuls-per-pair (last
     chunk also self-PVs one pair behind its exps); O^T evicted bf16 at the next
     chunk's top, tail (ones-matmul denominators, recip, bf16 transposes,
     tensor_scalar normalize) emitted at pair 6 two chunks later so the in-order
     PE queue never waits on the rowacc chain.
   - outputs alternate sync/gpsimd queues; the last two chunks split their DMA
     across queues so the final 512B-packet transfers land in parallel.
   - EXP activation table preloaded via a dummy [128,1] exp during ramp; b1 prep
     (transposes+projections) emitted at the tops of chunks 2-4 where the PE has
     exp-paced idle slots.
"""

import sys

sys.path.insert(0, "/opt/trn_rl_repo")

import numpy as np

import concourse.bass as bass
import concourse.tile as tile
from concourse import bacc, mybir
from concourse.bass_utils import run_bass_kernel_spmd
from concourse.masks import make_identity

S = 2048
B = 16
DH = 100
DK = 128
NCORES = 8
BPC = B // NCORES
F32 = mybir.dt.float32
BF16 = mybir.dt.bfloat16
FP16 = mybir.dt.float16
SCALE = 1.0 / float(np.sqrt(np.float32(DK)))

ST = S // 128     # 16 t-tiles of 128
NSC = S // 512    # 4 chunks of 512 per batch
XCOLS = ST * 2 * DH + 32  # [128, 3232]: 16 tiles x 200 cols + pad for b1/k15 window


def _attention_kernel(tc, out, x1, x2, wq, wk, wv):
    nc = tc.nc

    with (
        tc.tile_pool(name="const", bufs=1) as constp,
        tc.tile_pool(name="xn", bufs=2) as xnp,
        tc.tile_pool(name="xT", bufs=4) as xtp,
        tc.tile_pool(name="qk", bufs=4) as qkp,
        tc.tile_pool(name="vp", bufs=2) as vp,
        tc.tile_pool(name="et", bufs=2) as etp,
        tc.tile_pool(name="acc", bufs=2) as accp,
        tc.tile_pool(name="rowb", bufs=2) as rowbp,
        tc.tile_pool(name="rr", bufs=2) as rrp,
        tc.tile_pool(name="osb", bufs=2) as osbp,
        tc.tile_pool(name="osc", bufs=2) as oscp,
        tc.tile_pool(name="ps_st", bufs=2, space="PSUM") as psb,
        tc.tile_pool(name="ps_ot", bufs=2, space="PSUM") as psot,
        tc.tile_pool(name="ps_sc", bufs=2, space="PSUM") as pssc,
    ):
        # ---- identity first (gpsimd) so it doesn't queue behind DMA issues
        ident = constp.tile([128, 128], F32)
        make_identity(nc, ident)
        ident_bf = constp.tile([128, 128], BF16)
        nc.vector.tensor_copy(ident_bf, ident)

        # ---- x loads as bf16 tiles [128, 16x200]. DMA is packet-rate limited
        # (~800B packets, ~40-60GB/s per queue stream), so chunk-0's working set
        # (all of x2 + x1 g0) is scheduled by deadline across all three DMA
        # queues (sync/scalar HWDGE fp32 + DVE cast; gpsimd DGE casts inline).
        xn_tiles = {}
        for src_i in (0, 1):
            xn_tiles[src_i] = xnp.tile(
                [128, XCOLS], BF16, tag="xn", name=f"xn{src_i}"
            )

        def x_pair_src(src_ap, p):
            return src_ap[p * 256:(p + 1) * 256, :, :].rearrange(
                "(t q) b d -> q t (b d)", t=2
            )

        def x_group_src(src_ap, g):
            return src_ap[g * 512:(g + 1) * 512, :, :].rearrange(
                "(k p) b d -> p k (b d)", k=4
            )

        w_f32s = {}
        for wname, wap in (("wk", wk), ("wq", wq), ("wv", wv)):
            w_f32s[wname] = constp.tile([DH, DK], F32, name=f"{wname}_f32")
        # deadline-scheduled input streams (measured ~3.3-5us per 205KB pair
        # per queue): sync carries x2 p0/p2/p4/p7, scalar x2 p1/p5 (fp32
        # staging + DVE casts interleaved into the prep chain), gpsimd carries
        # x1 p0/p1, wv, x2 p3/p6 (DGE bf16 casts) and x1 groups 1-3.
        stg_x2 = constp.tile([128, 2400], F32, name="stg_x2")
        STAGED = {0: 0, 2: 400, 4: 800, 7: 1200, 1: 1600, 5: 2000}
        # critical x pairs lead each queue; weights ride second (their casts
        # aren't needed until the first projections ~2us later)
        nc.sync.dma_start(stg_x2[:, 0:400], x_pair_src(x2, 0))
        nc.scalar.dma_start(stg_x2[:, 1600:2000], x_pair_src(x2, 1))
        nc.gpsimd.dma_start(xn_tiles[0][:, 0:400], x_pair_src(x1, 0))
        nc.sync.dma_start(w_f32s["wk"], wk)
        nc.scalar.dma_start(w_f32s["wq"], wq)
        nc.gpsimd.dma_start(xn_tiles[0][:, 400:800], x_pair_src(x1, 1))
        nc.sync.dma_start(stg_x2[:, 400:800], x_pair_src(x2, 2))
        nc.gpsimd.dma_start(w_f32s["wv"], wv)
        nc.gpsimd.dma_start(xn_tiles[1][:, 1200:1600], x_pair_src(x2, 3))
        nc.sync.dma_start(stg_x2[:, 800:1200], x_pair_src(x2, 4))
        nc.scalar.dma_start(stg_x2[:, 2000:2400], x_pair_src(x2, 5))
        nc.gpsimd.dma_start(xn_tiles[1][:, 2400:2800], x_pair_src(x2, 6))
        nc.sync.dma_start(stg_x2[:, 1200:1600], x_pair_src(x2, 7))
        for g in range(1, 4):
            nc.gpsimd.dma_start(
                xn_tiles[0][:, g * 800:(g + 1) * 800], x_group_src(x1, g)
            )

        def emit_x2_cast(p):
            off = STAGED[p]
            nc.vector.tensor_copy(
                xn_tiles[1][:, p * 400:(p + 1) * 400],
                stg_x2[:, off:off + 400],
            )

        # ---- constants / casts
        ones_bf = constp.tile([128, 1], BF16)
        nc.vector.memset(ones_bf, 1.0)
        w_sbs = {}
        for wname in ("wk", "wq", "wv"):
            w_sbs[wname] = constp.tile([DH, DK], BF16, name=f"{wname}_sb")
        # all w casts are deferred to just before their first use: emitted here
        # they block the DVE counting semaphore (and everything queued behind)
        # until their DMAs land
        wq_sb, wk_sb, wv_sb = w_sbs["wq"], w_sbs["wk"], w_sbs["wv"]
        # preload the EXP activation table during ramp (dummy exp)
        dum = constp.tile([128, 1], F32, name="dum")
        nc.vector.memset(dum, 0.0)
        dum_o = constp.tile([128, 1], BF16, name="dum_o")
        nc.scalar.activation(dum_o, dum, mybir.ActivationFunctionType.Exp)
        # zero the pad window read by the b=1, k=15 transpose slice
        for src_i in (0, 1):
            nc.gpsimd.memset(xn_tiles[src_i][:, ST * 2 * DH:], 0.0)

        # persistent transposed/projected tensors
        xTs, qTs, kTs, vas = {}, {}, {}, {}
        for src_i in (0, 1):
            for b in range(BPC):
                xTs[(src_i, b)] = xtp.tile(
                    [128, S], BF16, tag="xT", name=f"xT_{src_i}_{b}"
                )
        for b in range(BPC):
            qTs[b] = qkp.tile([DK, S], BF16, tag="qk", name=f"qT_{b}")
            kTs[b] = qkp.tile([DK, S], BF16, tag="qk", name=f"kT_{b}")
            vas[b] = vp.tile([128, S], BF16, tag="v", name=f"vall_{b}")

        def emit_xt(src_i, b, t0, nt):
            """Transpose nt bf16 t-tiles on PE, evict psum->SBUF xT on DVE."""
            psq = pssc.tile([128, nt * 128], BF16, tag="sc",
                            name=f"xq_{src_i}_{b}_{t0}")
            xn = xn_tiles[src_i]
            for j in range(nt):
                tt = t0 + j
                c0 = tt * 2 * DH + b * DH
                nc.tensor.transpose(
                    psq[:, j * 128:(j + 1) * 128], xn[:, c0:c0 + 128], ident_bf
                )
            nc.vector.tensor_copy(
                xTs[(src_i, b)][:, t0 * 128:(t0 + nt) * 128], psq
            )

        def emit_proj(dstT, w_sb, xT, b, c0, ncols, eng=None):
            csl = slice(c0, c0 + ncols)
            pj = pssc.tile([128, ncols], F32, tag="sc", name=f"pj_{b}_{c0}")
            nc.tensor.matmul(pj, w_sb, xT[:DH, csl], start=True, stop=True)
            if eng is nc.scalar:
                nc.scalar.copy(dstT[:, csl], pj)
            else:
                nc.vector.tensor_copy(dstT[:, csl], pj)

        def emit_prep_v(b):
            x2T = xTs[(1, b)]
            for g in range(4):
                psv = pssc.tile([128, 512], F32, tag="sc", name=f"vg_{b}_{g}")
                for j in range(4):
                    tt = g * 4 + j
                    nc.tensor.matmul(
                        psv[:, j * 128:(j + 1) * 128],
                        x2T[:DH, tt * 128:(tt + 1) * 128],
                        wv_sb,
                        start=True, stop=True,
                    )
                nc.vector.tensor_copy(vas[b][:, g * 512:(g + 1) * 512], psv)

        def emit_tail_evict(ti, otp):
            """Free the psot buf early: O^T psum -> SBUF (bf16) on DVE."""
            ot_sb = osbp.tile([128, 512], BF16, tag="osb", name=f"otsb_{ti}")
            nc.vector.tensor_copy(ot_sb, otp)
            return ot_sb

        def emit_tail_rest(b, sc, rowacc, ot_sb, dma_engs=(nc.sync,)):
            rs_all = pssc.tile([128, 4], F32, tag="sc", name=f"rs_{b}_{sc}")
            for si in range(4):
                nc.tensor.matmul(
                    rs_all[:, si:si + 1],
                    rowacc[:, si * 128:(si + 1) * 128], ones_bf,
                    start=True, stop=True,
                )
            rr_all = rrp.tile([128, 4], F32, tag="rr", name=f"rr_{b}_{sc}")
            nc.vector.reciprocal(rr_all, rs_all)
            otr_all = pssc.tile([128, 512], BF16, tag="sc", name=f"otr_{b}_{sc}")
            osc_all = oscp.tile([128, 512], F32, tag="osc", name=f"osc_{b}_{sc}")
            # out DMAs are 512B-packet limited (~25GB/s per queue stream):
            # spread chunks across queues so the backlog drains in parallel.
            # With a 4-way engine list (drain tails), each si block's DMA is
            # issued right after its normalize so transfers start ASAP.
            n = len(dma_engs)
            kn = 4 // n
            for si in range(4):
                scol = slice(si * 128, (si + 1) * 128)
                nc.tensor.transpose(otr_all[:, scol], ot_sb[:, scol], ident_bf)
            for si in range(4):
                scol = slice(si * 128, (si + 1) * 128)
                nc.vector.tensor_scalar_mul(
                    osc_all[:, scol], otr_all[:, scol], rr_all[:, si:si + 1]
                )
                if kn == 1:
                    s0 = sc * 512 + si * 128
                    dst = out[s0:s0 + 128, b, :].rearrange(
                        "(k p) d -> p k d", k=1
                    )
                    dma_engs[si].dma_start(dst, osc_all[:, scol])
            if kn > 1:
                for i, eng in enumerate(dma_engs):
                    s0 = sc * 512 + i * kn * 128
                    dst = out[s0:s0 + kn * 128, b, :].rearrange(
                        "(k p) d -> p k d", k=kn
                    )
                    eng.dma_start(
                        dst, osc_all[:, i * kn * 128:(i + 1) * kn * 128]
                    )

        # ---- main loop: 8 chunks with lag-1 self-PV: chunk i's PV matmuls run
        # one pair behind its own exps; the final et pair's PV runs at chunk
        # i+1's pair 0, so each chunk's O^T psum is evictable at pair 1 of the
        # next chunk. Chunk 1 transitions (carries chunk 0's full PV as well,
        # since vall[0] only exists after chunk 0's prep finishes).
        items = [(b, sc) for b in range(BPC) for sc in range(NSC)]
        NI = len(items)
        ets, rowaccs, potps, ot_sbs, acchs = {}, {}, {}, {}, {}
        for idx, (b, sc) in enumerate(items):
            if idx == 4:
                # vall[1] is first read by PV(4) during chunk 5
                emit_prep_v(1)
            qT, kT = qTs[b], kTs[b]
            ssl = slice(sc * 512, (sc + 1) * 512)
            et = etp.tile([128, ST * 512], BF16, tag="et", name=f"et_{b}_{sc}")
            ets[idx] = et
            # prev-style PV: chunk idx-1's PV runs during this chunk; evict the
            # chunk idx-2 psum first so the psot buf is free before the claim
            if idx >= 2:
                ot_sbs[idx - 2] = emit_tail_evict(idx - 2, potps[idx - 2])
            if idx >= 1:
                potps[idx - 1] = psot.tile([128, 512], F32, tag="ot",
                                           name=f"ot_{idx - 1}")
            if idx == NI - 1:
                potps[idx] = psot.tile([128, 512], F32, tag="ot",
                                       name=f"ot_{idx}")
            incr = idx >= NI - 2
            if incr:
                acchs[idx] = accp.tile([128, 4096], FP16, tag="acc",
                                       name=f"acch_{b}_{sc}")
            for g in range(ST // 2):
                if idx == 0 and g == 0:
                    # ramp-critical b0 prep at tile-pair granularity; w casts
                    # just-in-time so they can't block the DVE queue
                    emit_x2_cast(0)
                    emit_xt(1, 0, 0, 2)
                    nc.vector.tensor_copy(wk_sb, w_f32s["wk"])
                    emit_proj(kTs[0], wk_sb, xTs[(1, 0)], 0, 0, 256)
                    emit_x2_cast(1)
                    emit_xt(1, 0, 2, 2)
                    emit_proj(kTs[0], wk_sb, xTs[(1, 0)], 0, 256, 256)
                    emit_xt(0, 0, 0, 2)
                    emit_xt(0, 0, 2, 2)
                    nc.vector.tensor_copy(wq_sb, w_f32s["wq"])
                    emit_proj(qTs[0], wq_sb, xTs[(0, 0)], 0, 0, 512,
                              eng=nc.scalar)
                ps = psb.tile([128, 1024], F32, tag="st", name=f"st_{b}_{sc}_{g}")
                for h in range(2):
                    tt = g * 2 + h
                    nc.tensor.matmul(
                        ps[:, h * 512:(h + 1) * 512],
                        kT[:, tt * 128:(tt + 1) * 128],
                        qT[:, ssl],
                        start=True, stop=True,
                    )
                nc.scalar.activation(
                    et[:, g * 1024:(g + 1) * 1024], ps,
                    mybir.ActivationFunctionType.Exp, scale=SCALE,
                )
                if idx == 0 and g <= 5:
                    # pair g+2's kT prep, emitted AFTER this pair's S^T/exp so
                    # waiting on mid-pair DMA arrivals can't block the stream
                    pr = g + 2
                    if pr in STAGED:
                        emit_x2_cast(pr)
                    emit_xt(1, 0, pr * 2, 2)
                    emit_proj(kTs[0], wk_sb, xTs[(1, 0)], 0, pr * 256, 256)
                # b1 prep interleaved pair-by-pair so its PE/DVE work can
                # never displace more than one pair's worth of the stream
                if idx == 2:
                    if g % 2 == 0:
                        emit_xt(1, 1, (g // 2) * 4, 4)
                    else:
                        emit_proj(kTs[1], wk_sb, xTs[(1, 1)], 1,
                                  (g // 2) * 512, 512)
                if idx == 3:
                    if g % 2 == 0:
                        emit_xt(0, 1, (g // 2) * 4, 4)
                    else:
                        emit_proj(qTs[1], wq_sb, xTs[(0, 1)], 1,
                                  (g // 2) * 512, 512)
                if idx >= 1:
                    # PV of the previous chunk
                    pv = vas[items[idx - 1][0]]
                    pet = ets[idx - 1]
                    for h in range(2):
                        ptt = g * 2 + h
                        nc.tensor.matmul(
                            potps[idx - 1],
                            pv[:, ptt * 128:(ptt + 1) * 128],
                            pet[:, ptt * 512:(ptt + 1) * 512],
                            start=(ptt == 0), stop=(ptt == ST - 1),
                        )
                if idx == NI - 1:
                    # last chunk: self-PV one pair behind the exps
                    for h in range(2):
                        tt = g * 2 + h
                        nc.tensor.matmul(
                            potps[idx],
                            vas[b][:, tt * 128:(tt + 1) * 128],
                            et[:, tt * 512:(tt + 1) * 512],
                            start=(tt == 0), stop=(tt == ST - 1),
                        )
                if incr:
                    # incremental rowsum on DVE to shorten the drain
                    if g == 3:
                        nc.vector.tensor_add(
                            acchs[idx][:, :2048], et[:, :2048], et[:, 2048:4096]
                        )
                    elif g == 5:
                        nc.vector.tensor_add(
                            acchs[idx][:, :2048], acchs[idx][:, :2048],
                            et[:, 4096:6144],
                        )
                    elif g == 7:
                        # tiles 12-13 are ready after pair 6's exp
                        nc.vector.tensor_add(
                            acchs[idx][:, :1024], acchs[idx][:, :1024],
                            et[:, 6144:7168],
                        )
                if g == 6 and idx - 2 in ot_sbs:
                    ti = idx - 2
                    eng = nc.sync if ti % 2 == 0 else nc.gpsimd
                    tb, tsc = items[ti]
                    emit_tail_rest(tb, tsc, rowaccs[ti], ot_sbs.pop(ti),
                                   dma_engs=(eng,))
            if idx == 0:
                for gp in range(1, 4):
                    emit_xt(0, 0, gp * 4, 4)
                    emit_proj(qTs[0], wq_sb, xTs[(0, 0)], 0, gp * 512, 512)
                nc.vector.tensor_copy(wv_sb, w_f32s["wv"])
                emit_prep_v(0)

            # rowsum: DVE tree, all 16-bit to keep the 2x perf mode
            rowacc = rowbp.tile([128, 512], BF16, tag="rowb", name=f"row_{b}_{sc}")
            rowaccs[idx] = rowacc
            if incr:
                acch = acchs[idx]
                nc.vector.tensor_add(
                    acch[:, 1024:2048], acch[:, 1024:2048], et[:, 7168:8192]
                )
                nc.vector.tensor_add(
                    acch[:, :1024], acch[:, :1024], acch[:, 1024:2048]
                )
                nc.vector.tensor_add(
                    rowacc, acch[:, :512], acch[:, 512:1024]
                )
            else:
                # L1/L2 on DVE (2x mode, 16-bit) so rowacc's producer chain is
                # mostly prompt; only the short L3/L4 ride on the slow Pool
                acch = accp.tile([128, 4096], FP16, tag="acc", name=f"acch_{b}_{sc}")
                nc.vector.tensor_add(acch, et[:, :4096], et[:, 4096:])
                nc.vector.tensor_add(acch[:, :2048], acch[:, :2048], acch[:, 2048:])
                nc.gpsimd.tensor_add(acch[:, :1024], acch[:, :1024],
                                     acch[:, 1024:2048])
                nc.gpsimd.tensor_add(rowacc, acch[:, :512], acch[:, 512:1024])
        # drain: tails of the final two chunks; out DMAs split across both
        # queues so the last transfers land in parallel
        li = NI - 1
        osb6 = emit_tail_evict(li - 1, potps[li - 1])
        osb7 = emit_tail_evict(li, potps[li])
        tb6, tsc6 = items[li - 1]
        emit_tail_rest(tb6, tsc6, rowaccs[li - 1], osb6,
                       dma_engs=(nc.sync, nc.scalar, nc.gpsimd, nc.sync))
        tb, tsc = items[li]
        emit_tail_rest(tb, tsc, rowaccs[li], osb7,
                       dma_engs=(nc.scalar, nc.gpsimd, nc.sync, nc.scalar))


_NC_CACHE = None


def _build():
    global _NC_CACHE
    if _NC_CACHE is not None:
        return _NC_CACHE
    nc = bacc.Bacc("TRN2", target_bir_lowering=False, debug=False, num_devices=NCORES)
    x1 = nc.dram_tensor("x_1", (S, BPC, DH), F32, kind="ExternalInput").ap()
    x2 = nc.dram_tensor("x_2", (S, BPC, DH), F32, kind="ExternalInput").ap()
    wq = nc.dram_tensor("Wq", (DH, DK), F32, kind="ExternalInput").ap()
    wk = nc.dram_tensor("Wk", (DH, DK), F32, kind="ExternalInput").ap()
    wv = nc.dram_tensor("Wv", (DH, DK), F32, kind="ExternalInput").ap()
    out = nc.dram_tensor("out", (S, BPC, DK), F32, kind="ExternalOutput").ap()
    with tile.TileContext(nc) as tc:
        _attention_kernel(tc, out, x1, x2, wq, wk, wv)
    nc.compile()
    _NC_CACHE = nc
    return nc


def _in_maps(x_1, x_2, Wq, Wk, Wv):
    maps = []
    for c in range(NCORES):
        bsl = slice(c * BPC, (c + 1) * BPC)
        maps.append({
            "x_1": np.ascontiguousarray(x_1[:, bsl, :], dtype=np.float32),
            "x_2": np.ascontiguousarray(x_2[:, bsl, :], dtype=np.float32),
            "Wq": np.asarray(Wq, dtype=np.float32),
            "Wk": np.asarray(Wk, dtype=np.float32),
            "Wv": np.asarray(Wv, dtype=np.float32),
        })
    return maps


def run(x_1, x_2, Wq, Wk, Wv, **spmd_kwargs):
    nc = _build()
    in_maps = _in_maps(x_1, x_2, Wq, Wk, Wv)
    last_err = None
    for _attempt in range(3):
        try:
            res = run_bass_kernel_spmd(
                nc, in_maps, core_ids=list(range(NCORES)), **spmd_kwargs
            )
            break
        except Exception as e:  # transient NRT_EXEC_UNIT_UNRECOVERABLE faults
            last_err = e
    else:
        raise last_err
    out = np.concatenate([res.results[c]["out"] for c in range(NCORES)], axis=1)
    return out, res


def kernel(x_1, x_2, Wq, Wk, Wv):
    out, _ = run(x_1, x_2, Wq, Wk, Wv)
    return out.astype(np.float32)


# revision 76
# speedup vs baseline: 1.0110x; 1.0110x over previous
"""Cross-attention Bass/Tile kernel for Trainium2, data-parallel over batch on 8 cores.

Problem (hardcoded): x_1 [2048,16,100], x_2 [2048,16,100], Wq/Wk/Wv [100,128], fp32.
  Q = x1 @ Wq; K = x2 @ Wk; V = x2 @ Wv  (per batch)
  out = softmax(Q K^T / sqrt(128)) @ V   -> [2048,16,128]

Sharding: batch dim split 8 ways (2 batches per core). Full inputs in, full output out.

Design notes (ACT-engine paced; ~104us vs 121us baseline):
  The per-core floor is the scalar/ACT engine: 2*S*S = 8.4M exps at 1 elem/lane/cycle
  @1.2GHz + ~260ns/instr bubble -> ~71.5us for 64 [128,1024] EXPs (measured 1117ns
  cadence). PE issue work (S^T + PV + prep) is ~65us. So ACT does NOTHING but the
  exps, and every other engine/queue is scheduled around keeping that stream dense:
   - hardware facts (trace-measured): LDWEIGHTS fully overlaps matmuls; bf16 matmul
     issues 1 col/cycle @2.4GHz (+163ns drain in reported durations); fp32
     transpose runs LOW_HIGH at ~1/4 rate (so transposes stay bf16); Pool/gpsimd
     cannot access PSUM and runs tensor ops at ~0.5 elem/cycle/lane; DVE 2x mode
     needs all-2-byte APs; DMA is packet-rate limited: 800B in-packets give
     ~40-60GB/s per queue stream, 512B out-packets ~25GB/s.
   - input: x2 (all of it) + x1 g0 gate chunk 0, ~2MB > queue rate, so pairs of
     t-tiles are deadline-scheduled across sync/scalar (fp32 staging + DVE casts
     interleaved into the prep chain) and gpsimd DGE-cast streams.
   - PSUM evictions (xT/qT/kT/V/O^T, all psum->SBUF) and tail muls live on DVE;
     rowsum tree: L1/L2 on DVE (2x, 16-bit), short L3/L4 on Pool; chunks 6-7 use
     an incremental DVE tree so the drain tails are not gated on Pool.
   - per chunk: S^T pair [128,1024] fp32 psum (2 matmuls) -> ACT exp*scale -> et
     bf16 SBUF; PV of the previous chunk interleaved 2-matmuls-per-pair (last
     chunk also self-PVs one pair behind its exps); O^T evicted bf16 at the next
     chunk's top, tail (ones-matmul denominators, recip, bf16 transposes,
     tensor_scalar normalize) emitted at pair 6 two chunks later so the in-order
     PE queue never waits on the rowacc chain.
   - outputs alternate sync/gpsimd queues; the last two chunks split their DMA
     across queues so the final 512B-packet transfers land in parallel.
   - EXP activation table preloaded via a dummy [128,1] exp during ramp; b1 prep
     (transposes+projections) emitted at the tops of chunks 2-4 where the PE has
     exp-paced idle slots.
"""

import sys

sys.path.insert(0, "/opt/trn_rl_repo")

import numpy as np

import concourse.bass as bass
import concourse.tile as tile
from concourse import bacc, mybir
from concourse.bass_utils import run_bass_kernel_spmd
from concourse.masks import make_identity

S = 2048
B = 16
DH = 100
DK = 128
NCORES = 8
BPC = B // NCORES
F32 = mybir.dt.float32
BF16 = mybir.dt.bfloat16
FP16 = mybir.dt.float16
SCALE = 1.0 / float(np.sqrt(np.float32(DK)))

ST = S // 128     # 16 t-tiles of 128
NSC = S // 512    # 4 chunks of 512 per batch
XCOLS = ST * 2 * DH + 32  # [128, 3232]: 16 tiles x 200 cols + pad for b1/k15 window


def _attention_kernel(tc, out, x1, x2, wq, wk, wv):
    nc = tc.nc

    with (
        tc.tile_pool(name="const", bufs=1) as constp,
        tc.tile_pool(name="xn", bufs=2) as xnp,
        tc.tile_pool(name="xT", bufs=4) as xtp,
        tc.tile_pool(name="qk", bufs=4) as qkp,
        tc.tile_pool(name="vp", bufs=2) as vp,
        tc.tile_pool(name="et", bufs=2) as etp,
        tc.tile_pool(name="acc", bufs=2) as accp,
        tc.tile_pool(name="rowb", bufs=2) as rowbp,
        tc.tile_pool(name="rr", bufs=2) as rrp,
        tc.tile_pool(name="osb", bufs=2) as osbp,
        tc.tile_pool(name="osc", bufs=2) as oscp,
        tc.tile_pool(name="ps_st", bufs=2, space="PSUM") as psb,
        tc.tile_pool(name="ps_ot", bufs=2, space="PSUM") as psot,
        tc.tile_pool(name="ps_sc", bufs=2, space="PSUM") as pssc,
    ):
        # ---- identity first (gpsimd) so it doesn't queue behind DMA issues
        ident = constp.tile([128, 128], F32)
        make_identity(nc, ident)
        ident_bf = constp.tile([128, 128], BF16)
        nc.vector.tensor_copy(ident_bf, ident)

        # ---- x loads as bf16 tiles [128, 16x200]. DMA is packet-rate limited
        # (~800B packets, ~40-60GB/s per queue stream), so chunk-0's working set
        # (all of x2 + x1 g0) is scheduled by deadline across all three DMA
        # queues (sync/scalar HWDGE fp32 + DVE cast; gpsimd DGE casts inline).
        xn_tiles = {}
        for src_i in (0, 1):
            xn_tiles[src_i] = xnp.tile(
                [128, XCOLS], BF16, tag="xn", name=f"xn{src_i}"
            )

        def x_pair_src(src_ap, p):
            return src_ap[p * 256:(p + 1) * 256, :, :].rearrange(
                "(t q) b d -> q t (b d)", t=2
            )

        def x_group_src(src_ap, g):
            return src_ap[g * 512:(g + 1) * 512, :, :].rearrange(
                "(k p) b d -> p k (b d)", k=4
            )

        w_f32s = {}
        for wname, wap in (("wk", wk), ("wq", wq), ("wv", wv)):
            w_f32s[wname] = constp.tile([DH, DK], F32, name=f"{wname}_f32")
        # deadline-scheduled input streams (measured ~3.3-5us per 205KB pair
        # per queue): sync carries x2 p0/p2/p4/p7, scalar x2 p1/p5 (fp32
        # staging + DVE casts interleaved into the prep chain), gpsimd carries
        # x1 p0/p1, wv, x2 p3/p6 (DGE bf16 casts) and x1 groups 1-3.
        stg_x2 = constp.tile([128, 2400], F32, name="stg_x2")
        STAGED = {0: 0, 2: 400, 4: 800, 7: 1200, 1: 1600, 5: 2000}
        # critical x pairs lead each queue; weights ride second (their casts
        # aren't needed until the first projections ~2us later)
        nc.sync.dma_start(stg_x2[:, 0:400], x_pair_src(x2, 0))
        nc.scalar.dma_start(stg_x2[:, 1600:2000], x_pair_src(x2, 1))
        nc.gpsimd.dma_start(xn_tiles[0][:, 0:400], x_pair_src(x1, 0))
        nc.sync.dma_start(w_f32s["wk"], wk)
        nc.scalar.dma_start(w_f32s["wq"], wq)
        nc.gpsimd.dma_start(xn_tiles[0][:, 400:800], x_pair_src(x1, 1))
        nc.sync.dma_start(stg_x2[:, 400:800], x_pair_src(x2, 2))
        nc.gpsimd.dma_start(w_f32s["wv"], wv)
        nc.gpsimd.dma_start(xn_tiles[1][:, 1200:1600], x_pair_src(x2, 3))
        nc.sync.dma_start(stg_x2[:, 800:1200], x_pair_src(x2, 4))
        nc.scalar.dma_start(stg_x2[:, 2000:2400], x_pair_src(x2, 5))
        nc.gpsimd.dma_start(xn_tiles[1][:, 2400:2800], x_pair_src(x2, 6))
        nc.sync.dma_start(stg_x2[:, 1200:1600], x_pair_src(x2, 7))
        for g in range(1, 4):
            nc.gpsimd.dma_start(
                xn_tiles[0][:, g * 800:(g + 1) * 800], x_group_src(x1, g)
            )

        def emit_x2_cast(p):
            off = STAGED[p]
            nc.vector.tensor_copy(
                xn_tiles[1][:, p * 400:(p + 1) * 400],
                stg_x2[:, off:off + 400],
            )

        # ---- constants / casts
        ones_bf = constp.tile([128, 1], BF16)
        nc.vector.memset(ones_bf, 1.0)
        w_sbs = {}
        for wname in ("wk", "wq", "wv"):
            w_sbs[wname] = constp.tile([DH, DK], BF16, name=f"{wname}_sb")
        # all w casts are deferred to just before their first use: emitted here
        # they block the DVE counting semaphore (and everything queued behind)
        # until their DMAs land
        wq_sb, wk_sb, wv_sb = w_sbs["wq"], w_sbs["wk"], w_sbs["wv"]
        # preload the EXP activation table during ramp (dummy exp)
        dum = constp.tile([128, 1], F32, name="dum")
        nc.vector.memset(dum, 0.0)
        dum_o = constp.tile([128, 1], BF16, name="dum_o")
        nc.scalar.activation(dum_o, dum, mybir.ActivationFunctionType.Exp)
        # zero the pad window read by the b=1, k=15 transpose slice
        for src_i in (0, 1):
            nc.gpsimd.memset(xn_tiles[src_i][:, ST * 2 * DH:], 0.0)

        # persistent transposed/projected tensors
        xTs, qTs, kTs, vas = {}, {}, {}, {}
        for src_i in (0, 1):
            for b in range(BPC):
                xTs[(src_i, b)] = xtp.tile(
                    [128, S], BF16, tag="xT", name=f"xT_{src_i}_{b}"
                )
        for b in range(BPC):
            qTs[b] = qkp.tile([DK, S], BF16, tag="qk", name=f"qT_{b}")
            kTs[b] = qkp.tile([DK, S], BF16, tag="qk", name=f"kT_{b}")
            vas[b] = vp.tile([128, S], BF16, tag="v", name=f"vall_{b}")

        def emit_xt(src_i, b, t0, nt):
            """Transpose nt bf16 t-tiles on PE, evict psum->SBUF xT on DVE."""
            psq = pssc.tile([128, nt * 128], BF16, tag="sc",
                            name=f"xq_{src_i}_{b}_{t0}")
            xn = xn_tiles[src_i]
            for j in range(nt):
                tt = t0 + j
                c0 = tt * 2 * DH + b * DH
                nc.tensor.transpose(
                    psq[:, j * 128:(j + 1) * 128], xn[:, c0:c0 + 128], ident_bf
                )
            nc.vector.tensor_copy(
                xTs[(src_i, b)][:, t0 * 128:(t0 + nt) * 128], psq
            )

        def emit_proj(dstT, w_sb, xT, b, c0, ncols, eng=None):
            csl = slice(c0, c0 + ncols)
            pj = pssc.tile([128, ncols], F32, tag="sc", name=f"pj_{b}_{c0}")
            nc.tensor.matmul(pj, w_sb, xT[:DH, csl], start=True, stop=True)
            if eng is nc.scalar:
                nc.scalar.copy(dstT[:, csl], pj)
            else:
                nc.vector.tensor_copy(dstT[:, csl], pj)

        def emit_prep_v(b):
            x2T = xTs[(1, b)]
            for g in range(4):
                psv = pssc.tile([128, 512], F32, tag="sc", name=f"vg_{b}_{g}")
                for j in range(4):
                    tt = g * 4 + j
                    nc.tensor.matmul(
                        psv[:, j * 128:(j + 1) * 128],
                        x2T[:DH, tt * 128:(tt + 1) * 128],
                        wv_sb,
                        start=True, stop=True,
                    )
                nc.vector.tensor_copy(vas[b][:, g * 512:(g + 1) * 512], psv)

        def emit_tail_evict(ti, otp):
            """Free the psot buf early: O^T psum -> SBUF (bf16) on DVE."""
            ot_sb = osbp.tile([128, 512], BF16, tag="osb", name=f"otsb_{ti}")
            nc.vector.tensor_copy(ot_sb, otp)
            return ot_sb

        def emit_tail_rest(b, sc, rowacc, ot_sb, dma_engs=(nc.sync,)):
            rs_all = pssc.tile([128, 4], F32, tag="sc", name=f"rs_{b}_{sc}")
            for si in range(4):
                nc.tensor.matmul(
                    rs_all[:, si:si + 1],
                    rowacc[:, si * 128:(si + 1) * 128], ones_bf,
                    start=True, stop=True,
                )
            rr_all = rrp.tile([128, 4], F32, tag="rr", name=f"rr_{b}_{sc}")
            nc.vector.reciprocal(rr_all, rs_all)
            otr_all = pssc.tile([128, 512], BF16, tag="sc", name=f"otr_{b}_{sc}")
            osc_all = oscp.tile([128, 512], F32, tag="osc", name=f"osc_{b}_{sc}")
            # out DMAs are 512B-packet limited (~25GB/s per queue stream):
            # spread chunks across queues so the backlog drains in parallel.
            # With a 4-way engine list (drain tails), each si block's DMA is
            # issued right after its normalize so transfers start ASAP.
            n = len(dma_engs)
            kn = 4 // n
            for si in range(4):
                scol = slice(si * 128, (si + 1) * 128)
                nc.tensor.transpose(otr_all[:, scol], ot_sb[:, scol], ident_bf)
            for si in range(4):
                scol = slice(si * 128, (si + 1) * 128)
                nc.vector.tensor_scalar_mul(
                    osc_all[:, scol], otr_all[:, scol], rr_all[:, si:si + 1]
                )
                if kn == 1:
                    s0 = sc * 512 + si * 128
                    dst = out[s0:s0 + 128, b, :].rearrange(
                        "(k p) d -> p k d", k=1
                    )
                    dma_engs[si].dma_start(dst, osc_all[:, scol])
            if kn > 1:
                for i, eng in enumerate(dma_engs):
                    s0 = sc * 512 + i * kn * 128
                    dst = out[s0:s0 + kn * 128, b, :].rearrange(
                        "(k p) d -> p k d", k=kn
                    )
                    eng.dma_start(
                        dst, osc_all[:, i * kn * 128:(i + 1) * kn * 128]
                    )

        # ---- main loop: 8 chunks with lag-1 self-PV: chunk i's PV matmuls run
        # one pair behind its own exps; the final et pair's PV runs at chunk
        # i+1's pair 0, so each chunk's O^T psum is evictable at pair 1 of the
        # next chunk. Chunk 1 transitions (carries chunk 0's full PV as well,
        # since vall[0] only exists after chunk 0's prep finishes).
        items = [(b, sc) for b in range(BPC) for sc in range(NSC)]
        NI = len(items)
        ets, rowaccs, potps, ot_sbs, acchs = {}, {}, {}, {}, {}
        for idx, (b, sc) in enumerate(items):
            # b1 prep at chunk tops: PE fills exp-paced idle slots and the DVE
            # evictions drain ahead of this chunk's tail work
            if idx == 2:
                for g2 in range(4):
                    emit_xt(1, 1, g2 * 4, 4)
                for c in range(NSC):
                    emit_proj(kTs[1], wk_sb, xTs[(1, 1)], 1, c * 512, 512)
            if idx == 3:
                for g2 in range(4):
                    emit_xt(0, 1, g2 * 4, 4)
                for c in range(NSC):
                    emit_proj(qTs[1], wq_sb, xTs[(0, 1)], 1, c * 512, 512)
            if idx == 4:
                # vall[1] is first read by PV(4) during chunk 5
                emit_prep_v(1)
            qT, kT = qTs[b], kTs[b]
            ssl = slice(sc * 512, (sc + 1) * 512)
            et = etp.tile([128, ST * 512], BF16, tag="et", name=f"et_{b}_{sc}")
            ets[idx] = et
            # prev-style PV: chunk idx-1's PV runs during this chunk; evict the
            # chunk idx-2 psum first so the psot buf is free before the claim
            if idx >= 2:
                ot_sbs[idx - 2] = emit_tail_evict(idx - 2, potps[idx - 2])
            if idx >= 1:
                potps[idx - 1] = psot.tile([128, 512], F32, tag="ot",
                                           name=f"ot_{idx - 1}")
            if idx == NI - 1:
                potps[idx] = psot.tile([128, 512], F32, tag="ot",
                                       name=f"ot_{idx}")
            incr = idx >= NI - 2
            if incr:
                acchs[idx] = accp.tile([128, 4096], FP16, tag="acc",
                                       name=f"acch_{b}_{sc}")
            for g in range(ST // 2):
                if idx == 0 and g == 0:
                    # ramp-critical b0 prep at tile-pair granularity; w casts
                    # just-in-time so they can't block the DVE queue
                    emit_x2_cast(0)
                    emit_xt(1, 0, 0, 2)
                    nc.vector.tensor_copy(wk_sb, w_f32s["wk"])
                    emit_proj(kTs[0], wk_sb, xTs[(1, 0)], 0, 0, 256)
                    emit_x2_cast(1)
                    emit_xt(1, 0, 2, 2)
                    emit_proj(kTs[0], wk_sb, xTs[(1, 0)], 0, 256, 256)
                    emit_xt(0, 0, 0, 2)
                    emit_xt(0, 0, 2, 2)
                    nc.vector.tensor_copy(wq_sb, w_f32s["wq"])
                    emit_proj(qTs[0], wq_sb, xTs[(0, 0)], 0, 0, 512,
                              eng=nc.scalar)
                ps = psb.tile([128, 1024], F32, tag="st", name=f"st_{b}_{sc}_{g}")
                for h in range(2):
                    tt = g * 2 + h
                    nc.tensor.matmul(
                        ps[:, h * 512:(h + 1) * 512],
                        kT[:, tt * 128:(tt + 1) * 128],
                        qT[:, ssl],
                        start=True, stop=True,
                    )
                nc.scalar.activation(
                    et[:, g * 1024:(g + 1) * 1024], ps,
                    mybir.ActivationFunctionType.Exp, scale=SCALE,
                )
                if idx == 0 and g <= 5:
                    # pair g+2's kT prep, emitted AFTER this pair's S^T/exp so
                    # waiting on mid-pair DMA arrivals can't block the stream
                    pr = g + 2
                    if pr in STAGED:
                        emit_x2_cast(pr)
                    emit_xt(1, 0, pr * 2, 2)
                    emit_proj(kTs[0], wk_sb, xTs[(1, 0)], 0, pr * 256, 256)

                if idx >= 1:
                    # PV of the previous chunk
                    pv = vas[items[idx - 1][0]]
                    pet = ets[idx - 1]
                    for h in range(2):
                        ptt = g * 2 + h
                        nc.tensor.matmul(
                            potps[idx - 1],
                            pv[:, ptt * 128:(ptt + 1) * 128],
                            pet[:, ptt * 512:(ptt + 1) * 512],
                            start=(ptt == 0), stop=(ptt == ST - 1),
                        )
                if idx == NI - 1:
                    # last chunk: self-PV one pair behind the exps
                    for h in range(2):
                        tt = g * 2 + h
                        nc.tensor.matmul(
                            potps[idx],
                            vas[b][:, tt * 128:(tt + 1) * 128],
                            et[:, tt * 512:(tt + 1) * 512],
                            start=(tt == 0), stop=(tt == ST - 1),
                        )
                if incr:
                    # incremental rowsum on DVE to shorten the drain
                    if g == 3:
                        nc.vector.tensor_add(
                            acchs[idx][:, :2048], et[:, :2048], et[:, 2048:4096]
                        )
                    elif g == 5:
                        nc.vector.tensor_add(
                            acchs[idx][:, :2048], acchs[idx][:, :2048],
                            et[:, 4096:6144],
                        )
                    elif g == 7:
                        # tiles 12-13 are ready after pair 6's exp
                        nc.vector.tensor_add(
                            acchs[idx][:, :1024], acchs[idx][:, :1024],
                            et[:, 6144:7168],
                        )
                if g == 6 and idx - 2 in ot_sbs:
                    ti = idx - 2
                    eng = nc.sync if ti % 2 == 0 else nc.gpsimd
                    tb, tsc = items[ti]
                    emit_tail_rest(tb, tsc, rowaccs[ti], ot_sbs.pop(ti),
                                   dma_engs=(eng,))
            if idx == 0:
                for gp in range(1, 4):
                    emit_xt(0, 0, gp * 4, 4)
                    emit_proj(qTs[0], wq_sb, xTs[(0, 0)], 0, gp * 512, 512)
                nc.vector.tensor_copy(wv_sb, w_f32s["wv"])
                emit_prep_v(0)

            # rowsum: DVE tree, all 16-bit to keep the 2x perf mode
            rowacc = rowbp.tile([128, 512], BF16, tag="rowb", name=f"row_{b}_{sc}")
            rowaccs[idx] = rowacc
            if incr:
                acch = acchs[idx]
                nc.vector.tensor_add(
                    acch[:, 1024:2048], acch[:, 1024:2048], et[:, 7168:8192]
                )
                nc.vector.tensor_add(
                    acch[:, :1024], acch[:, :1024], acch[:, 1024:2048]
                )
                nc.vector.tensor_add(
                    rowacc, acch[:, :512], acch[:, 512:1024]
                )
            else:
                # L1/L2 on DVE (2x mode, 16-bit) so rowacc's producer chain is
                # mostly prompt; only the short L3/L4 ride on the slow Pool
                acch = accp.tile([128, 4096], FP16, tag="acc", name=f"acch_{b}_{sc}")
                nc.vector.tensor_add(acch, et[:, :4096], et[:, 4096:])
                nc.vector.tensor_add(acch[:, :2048], acch[:, :2048], acch[:, 2048:])
                nc.gpsimd.tensor_add(acch[:, :1024], acch[:, :1024],
                                     acch[:, 1024:2048])
                nc.gpsimd.tensor_add(rowacc, acch[:, :512], acch[:, 512:1024])
        # drain: tails of the final two chunks; out DMAs split across both
        # queues so the last transfers land in parallel
        li = NI - 1
        osb6 = emit_tail_evict(li - 1, potps[li - 1])
        osb7 = emit_tail_evict(li, potps[li])
        tb6, tsc6 = items[li - 1]
        emit_tail_rest(tb6, tsc6, rowaccs[li - 1], osb6,
                       dma_engs=(nc.sync, nc.scalar, nc.gpsimd, nc.sync))
        tb, tsc = items[li]
        emit_tail_rest(tb, tsc, rowaccs[li], osb7,
                       dma_engs=(nc.scalar, nc.gpsimd, nc.sync, nc.scalar))


_NC_CACHE = None


def _build():
    global _NC_CACHE
    if _NC_CACHE is not None:
        return _NC_CACHE
    nc = bacc.Bacc("TRN2", target_bir_lowering=False, debug=False, num_devices=NCORES)
    x1 = nc.dram_tensor("x_1", (S, BPC, DH), F32, kind="ExternalInput").ap()
    x2 = nc.dram_tensor("x_2", (S, BPC, DH), F32, kind="ExternalInput").ap()
    wq = nc.dram_tensor("Wq", (DH, DK), F32, kind="ExternalInput").ap()
    wk = nc.dram_tensor("Wk", (DH, DK), F32, kind="ExternalInput").ap()
    wv = nc.dram_tensor("Wv", (DH, DK), F32, kind="ExternalInput").ap()
    out = nc.dram_tensor("out", (S, BPC, DK), F32, kind="ExternalOutput").ap()
    with tile.TileContext(nc) as tc:
        _attention_kernel(tc, out, x1, x2, wq, wk, wv)
    nc.compile()
    _NC_CACHE = nc
    return nc


def _in_maps(x_1, x_2, Wq, Wk, Wv):
    maps = []
    for c in range(NCORES):
        bsl = slice(c * BPC, (c + 1) * BPC)
        maps.append({
            "x_1": np.ascontiguousarray(x_1[:, bsl, :], dtype=np.float32),
            "x_2": np.ascontiguousarray(x_2[:, bsl, :], dtype=np.float32),
            "Wq": np.asarray(Wq, dtype=np.float32),
            "Wk": np.asarray(Wk, dtype=np.float32),
            "Wv": np.asarray(Wv, dtype=np.float32),
        })
    return maps


def run(x_1, x_2, Wq, Wk, Wv, **spmd_kwargs):
    nc = _build()
    in_maps = _in_maps(x_1, x_2, Wq, Wk, Wv)
    last_err = None
    for _attempt in range(3):
        try:
            res = run_bass_kernel_spmd(
                nc, in_maps, core_ids=list(range(NCORES)), **spmd_kwargs
            )
            break
        except Exception as e:  # transient NRT_EXEC_UNIT_UNRECOVERABLE faults
            last_err = e
    else:
        raise last_err
    out = np.concatenate([res.results[c]["out"] for c in range(NCORES)], axis=1)
    return out, res


def kernel(x_1, x_2, Wq, Wk, Wv):
    out, _ = run(x_1, x_2, Wq, Wk, Wv)
    return out.astype(np.float32)
